# revision 15
# baseline (speedup 1.0000x reference)
"""NetsVocab per-word MLP kernel for 8 Trainium2 NeuronCores.

Math (per active word w of 16, per box b of 8192):
    h1 = relu(x @ W1[w] + b1[w])        # [B,4096] @ [4096,64]
    h2 = relu(h1 @ W2[w] + b2[w])       # [B,64] @ [64,32]
    l  = h2 @ W3[w] + b3[w]             # [B,32] @ [32]
    out[b] = prod_w sigmoid(l[w,b])

Strategy: data-parallel over boxes (1024 per core), word weights gathered
on host (W1[words] etc.), replicated to all cores. Layer-1 dominates
(8.6 GFLOP/core) and runs as bf16 PE matmuls with fp32 PSUM accumulation.
Layers 2/3 use host-packed block-diagonal weights (2 words per 128/64
partitions) so they are plain full-K matmuls. The 16-sigmoid product is a
pairwise DVE multiply tree; the final cross-partition (2->1) step goes
through one tiny SBUF->SBUF DMA since compute engines cannot read from a
non-32-aligned base partition.

Layouts (per core): DMA descriptors are generated per contiguous
per-partition run, and descriptor generation (~10 ns each) is the DMA
bottleneck for small runs — so the big tensors are host-packed
partition-major so each partition's whole k-range is one contiguous run
(16-32 KiB descriptors instead of 1-2 KiB).

    xT   [128, KT, 1024] bf16  [p, k, b] <-> x[c*1024+b, k*128+p]
    w1   [2, 128, KT, 512] bf16 [half, p, k, col] <-> W1cat[k*128+p, half*512+col]
    w2   [128, 8, 64]  bf16  per m-tile block-diag: 2 words' [64,32] blocks
    w3   [64, 8, 2]    bf16  per m-tile block-diag: 2 words' [32] columns
    b1   [128, 8] f32, b2 [64, 8] f32, b3 [2, 8] f32 (bias per m-tile col)
    out  [1, 1024] f32
"""

import os

import numpy as np
import ml_dtypes

import concourse.bass as bass
import concourse.tile as tile
from concourse import bacc
from concourse import mybir
from concourse.bass import ts
from concourse.bass_utils import run_bass_kernel_spmd

BF16 = mybir.dt.bfloat16
F32 = mybir.dt.float32
AF = mybir.ActivationFunctionType

N_CORES = 8
B = 8192            # total boxes
BC = B // N_CORES   # boxes per core (1024)
F = 4096            # features
NW = 16             # active words
H1 = 64
H2 = 32
KT = F // 128       # 32 k-tiles
MT = NW * H1 // 128  # 8 m-tiles (wh = w*64+h, 2 words per tile)
NT = BC // 512      # 2 n-tiles of 512 boxes

LAST_RESULTS = None  # BassKernelResults of the most recent run (for test.py)


def build_nc():
    nc = bacc.Bacc("TRN2", target_bir_lowering=False, debug=False)

    xT_d = nc.dram_tensor("xT", [128, KT, BC], BF16, kind="ExternalInput")
    w1_d = nc.dram_tensor("w1", [2, 128, KT, 512], BF16, kind="ExternalInput")
    w2_d = nc.dram_tensor("w2", [128, MT, H1], BF16, kind="ExternalInput")
    w3_d = nc.dram_tensor("w3", [64, MT, 2], BF16, kind="ExternalInput")
    b1_d = nc.dram_tensor("b1", [128, MT], F32, kind="ExternalInput")
    b2_d = nc.dram_tensor("b2", [64, MT], F32, kind="ExternalInput")
    b3_d = nc.dram_tensor("b3", [2, MT], F32, kind="ExternalInput")
    out_d = nc.dram_tensor("out", [1, BC], F32, kind="ExternalOutput")

    with tile.TileContext(nc) as tc:
        with (
            tc.tile_pool(name="big", bufs=1) as big,
            tc.tile_pool(name="smalls", bufs=1) as smalls,
            tc.tile_pool(name="h1p", bufs=4) as h1p,
            tc.tile_pool(name="h2p", bufs=4) as h2p,
            tc.tile_pool(name="sigp", bufs=3) as sigp,
            tc.tile_pool(name="prodp", bufs=1) as prodp,
            tc.tile_pool(name="accp", bufs=6, space="PSUM") as accp,
            tc.tile_pool(name="ps2p", bufs=1, space="PSUM") as ps2p,
            tc.tile_pool(name="ps3p", bufs=1, space="PSUM") as ps3p,
        ):
            w2_sb = smalls.tile([128, MT, H1], BF16, tag="w2", name="w2_sb")
            w3_sb = smalls.tile([64, MT, 2], BF16, tag="w3", name="w3_sb")
            b1_sb = smalls.tile([128, MT], F32, tag="b1", name="b1_sb")
            b2_sb = smalls.tile([64, MT], F32, tag="b2", name="b2_sb")
            b3_sb = smalls.tile([2, MT], F32, tag="b3", name="b3_sb")

            xT_sb = big.tile([128, KT, BC], BF16, tag="xT", name="xT_sb")
            w1_sb = big.tile([128, 2, KT, 512], BF16, tag="w1", name="w1_sb")

            # Chunked loads, 4 k-tiles per chunk (xT 1 MiB / w1-half 0.5 MiB,
            # per-partition runs of 8/4 KiB - big enough that descriptor
            # generation outruns the wire). Interleave so the (mg0, n0)
            # k-loop starts after just the first 1.5 MiB; w1's second half
            # arrives while mg0 computes. Small constants ride along after
            # the first wave (not needed until the first epilogue).
            # Progressive chunk sizes: small first waves so the k-loop can
            # start after ~1.5 MiB, then large chunks whose big
            # per-partition runs keep descriptor generation off the
            # critical path.
            waves = [(0, 4), (4, 4), (8, 8), (16, 16)]   # (k0, nk)
            for wi, (k0, nk) in enumerate(waves):
                nc.sync.dma_start(
                    out=w1_sb[:, 0, k0:k0 + nk, :], in_=w1_d[0, :, k0:k0 + nk, :]
                )
                nc.sync.dma_start(
                    out=xT_sb[:, k0:k0 + nk, :], in_=xT_d[:, k0:k0 + nk, :]
                )
                if wi == 0:
                    nc.sync.dma_start(out=w2_sb, in_=w2_d[:])
                    nc.sync.dma_start(out=w3_sb, in_=w3_d[:])
                    nc.sync.dma_start(out=b1_sb, in_=b1_d[:])
                    nc.sync.dma_start(out=b2_sb, in_=b2_d[:])
                    nc.sync.dma_start(out=b3_sb, in_=b3_d[:])
            for c in range(2):
                nc.sync.dma_start(
                    out=w1_sb[:, 1, ts(c, 16), :], in_=w1_d[1, :, ts(c, 16), :]
                )

            # Running product over the 8 word-pairs: prod[p, b] accumulates
            # prod_m sigmoid(logits) for pair-slot p (word 2m+p).
            prod = prodp.tile([2, BC], F32, tag="prod", name="prod")

            def l1_matmul(acc, m, n, k):
                nc.tensor.matmul(
                    acc,
                    w1_sb[:, m // 4, k, ts(m % 4, 128)],
                    xT_sb[:, k, ts(n, 512)],
                    start=(k == 0),
                    stop=(k == KT - 1),
                )

            def epilogue(m, n, acc):
                h1_t = h1p.tile([128, 512], BF16, tag="h1", name=f"h1_{m}_{n}")
                nc.scalar.activation(
                    h1_t, acc, AF.Relu, bias=b1_sb[:, m:m + 1]
                )
                ps2 = ps2p.tile([H1, 512], F32, tag="ps2", name=f"ps2_{m}_{n}")
                nc.tensor.matmul(ps2, w2_sb[:, m, :], h1_t, start=True, stop=True)
                h2_t = h2p.tile([H1, 512], BF16, tag="h2", name=f"h2_{m}_{n}")
                nc.scalar.activation(
                    h2_t, ps2, AF.Relu, bias=b2_sb[:, m:m + 1]
                )
                ps3 = ps3p.tile([2, 512], F32, tag="ps3", name=f"ps3_{m}_{n}")
                nc.tensor.matmul(ps3, w3_sb[:, m, :], h2_t, start=True, stop=True)
                if m == 0:
                    nc.scalar.activation(
                        prod[:, ts(n, 512)], ps3, AF.Sigmoid,
                        bias=b3_sb[:, m:m + 1],
                    )
                else:
                    sig_t = sigp.tile(
                        [2, 512], F32, tag="sig", name=f"sig_{m}_{n}"
                    )
                    nc.scalar.activation(
                        sig_t, ps3, AF.Sigmoid, bias=b3_sb[:, m:m + 1]
                    )
                    nc.vector.tensor_mul(
                        prod[:, ts(n, 512)], prod[:, ts(n, 512)], sig_t
                    )

            # Pass 1 (DMA-paced): k-outer over m=0..3 so each k-tile is
            # consumed as its chunk lands.
            accs = {
                m: accp.tile([128, 512], F32, tag="acc", name=f"acc_p1_{m}")
                for m in range(4)
            }
            for k in range(KT):
                for m in range(4):
                    l1_matmul(accs[m], m, 0, k)
            for m in range(4):
                epilogue(m, 0, accs[m])

            # Remaining passes (everything resident): m-serial so each
            # m-tile's epilogue overlaps the next m-tile's k-loop; the last
            # pass's epilogues then mostly hide under PE work.
            for ms_, n in ((range(4), 1), (range(4, 8), 0), (range(4, 8), 1)):
                for m in ms_:
                    acc = accp.tile(
                        [128, 512], F32, tag="acc", name=f"acc_{m}_{n}"
                    )
                    for k in range(KT):
                        l1_matmul(acc, m, n, k)
                    epilogue(m, n, acc)

            # Combine the two pair-slots: engines can't read base partition 1,
            # so bounce partition 1 to a fresh tile via SBUF->SBUF DMA.
            r1 = prodp.tile([1, BC], F32, tag="r1", name="r1")
            nc.sync.dma_start(out=r1, in_=prod[1:2, :])
            out_sb = prodp.tile([1, BC], F32, tag="outsb", name="out_sb")
            nc.vector.tensor_mul(out_sb, prod[0:1, :], r1)
            nc.sync.dma_start(out=out_d[:], in_=out_sb)

    nc.compile()
    return nc


_NC_CACHE = None


def _get_nc():
    global _NC_CACHE
    if _NC_CACHE is None:
        _NC_CACHE = build_nc()
    return _NC_CACHE


def _pack_inputs(x, words, W1, b1, W2, b2, W3, b3):
    bf = ml_dtypes.bfloat16
    words = np.asarray(words).astype(np.int64)

    w1g = np.asarray(W1)[words]                     # [16, 4096, 64]
    w1cat = w1g.transpose(1, 0, 2).reshape(F, NW * H1).astype(bf)  # [4096, 1024]
    # -> [half, p, k, col]: partition-major so each partition's whole
    # k-range is one contiguous DMA run.
    w1p = np.ascontiguousarray(
        w1cat.reshape(KT, 128, 2, 512).transpose(2, 1, 0, 3)
    )                                               # [2, 128, 32, 512]
    b1cat = np.asarray(b1)[words].reshape(NW * H1)  # [1024]
    b1p = np.ascontiguousarray(b1cat.reshape(MT, 128).T).astype(np.float32)

    w2g = np.asarray(W2)[words]                     # [16, 64, 32]
    w2blk = np.zeros((MT, 128, H1), np.float32)
    for t in range(MT):
        w2blk[t, 0:64, 0:32] = w2g[2 * t]
        w2blk[t, 64:128, 32:64] = w2g[2 * t + 1]
    w2p = np.ascontiguousarray(w2blk.transpose(1, 0, 2)).astype(bf)  # [128,8,64]
    b2g = np.asarray(b2)[words]                     # [16, 32]
    b2blk = np.zeros((MT, 64), np.float32)
    for t in range(MT):
        b2blk[t, 0:32] = b2g[2 * t]
        b2blk[t, 32:64] = b2g[2 * t + 1]
    b2p = np.ascontiguousarray(b2blk.T).astype(np.float32)           # [64, 8]

    w3g = np.asarray(W3)[words]                     # [16, 32]
    w3blk = np.zeros((MT, 64, 2), np.float32)
    for t in range(MT):
        w3blk[t, 0:32, 0] = w3g[2 * t]
        w3blk[t, 32:64, 1] = w3g[2 * t + 1]
    w3p = np.ascontiguousarray(w3blk.transpose(1, 0, 2)).astype(bf)  # [64, 8, 2]
    b3g = np.asarray(b3)[words]                     # [16]
    b3blk = b3g.reshape(MT, 2)
    b3p = np.ascontiguousarray(b3blk.T).astype(np.float32)           # [2, 8]

    x = np.asarray(x, dtype=np.float32)
    shared = {"w1": w1p, "w2": w2p, "w3": w3p, "b1": b1p, "b2": b2p, "b3": b3p}
    in_maps = []
    for c in range(N_CORES):
        # [p, k, b] partition-major (one contiguous 64 KiB run per partition)
        xT_c = np.ascontiguousarray(
            x[c * BC:(c + 1) * BC, :].astype(bf).T.reshape(KT, 128, BC)
            .transpose(1, 0, 2)
        )
        in_maps.append({"xT": xT_c, **shared})
    return in_maps


def _enable_trace():
    """Register the axon NTFF profile hook (the image's antenv lacks
    axon_hooks, so boot degraded silently) and disable artifact upload."""
    import sys
    import types
    import antenv
    from concourse import bass_utils as bu

    if "antenv.axon_hooks" not in sys.modules:
        mod = types.ModuleType("antenv.axon_hooks")
        mod._hook = None

        def set_axon_ntff_profile_hook(h):
            mod._hook = h

        def get_axon_ntff_profile_hook():
            return mod._hook

        mod.set_axon_ntff_profile_hook = set_axon_ntff_profile_hook
        mod.get_axon_ntff_profile_hook = get_axon_ntff_profile_hook
        sys.modules["antenv.axon_hooks"] = mod
        antenv.axon_hooks = mod

        from trn_agent_boot.trn_boot import _ntff_profile_via_ctypes

        set_axon_ntff_profile_hook(
            _ntff_profile_via_ctypes("/opt/axon/libaxon_pjrt.so")
        )

    bu.upload_artifacts = lambda tmpdir: tmpdir


def kernel(nBBox, x, words, W1, b1, W2, b2, W3, b3):
    global LAST_RESULTS
    nc = _get_nc()
    in_maps = _pack_inputs(x, words, W1, b1, W2, b2, W3, b3)
    trace = bool(int(os.environ.get("KERNEL_TRACE", "0")))
    if trace:
        _enable_trace()
    res = run_bass_kernel_spmd(
        nc, in_maps, core_ids=list(range(N_CORES)), trace=trace
    )
    LAST_RESULTS = res
    out = np.concatenate(
        [res.results[c]["out"].reshape(BC) for c in range(N_CORES)]
    )
    return out.astype(np.float32)[:, None]


# revision 18
# speedup vs baseline: 1.0646x; 1.0646x over previous
"""NetsVocab per-word MLP kernel for 8 Trainium2 NeuronCores.

Math (per active word w of 16, per box b of 8192):
    h1 = relu(x @ W1[w] + b1[w])        # [B,4096] @ [4096,64]
    h2 = relu(h1 @ W2[w] + b2[w])       # [B,64] @ [64,32]
    l  = h2 @ W3[w] + b3[w]             # [B,32] @ [32]
    out[b] = prod_w sigmoid(l[w,b])

Strategy: data-parallel over boxes (1024 per core), word weights gathered
on host (W1[words] etc.), replicated to all cores. Layer-1 dominates
(8.6 GFLOP/core) and runs as bf16 PE matmuls with fp32 PSUM accumulation.
Layers 2/3 use host-packed block-diagonal weights (2 words per 128/64
partitions) so they are plain full-K matmuls. The 16-sigmoid product is a
pairwise DVE multiply tree; the final cross-partition (2->1) step goes
through one tiny SBUF->SBUF DMA since compute engines cannot read from a
non-32-aligned base partition.

Layouts (per core): DMA descriptors are generated per contiguous
per-partition run, and descriptor generation (~10 ns each) is the DMA
bottleneck for small runs — so the big tensors are host-packed
partition-major so each partition's whole k-range is one contiguous run
(16-32 KiB descriptors instead of 1-2 KiB).

    xT   [128, KT, 1024] bf16  [p, k, b] <-> x[c*1024+b, k*128+p]
    w1   [2, 128, KT, 512] bf16 [half, p, k, col] <-> W1cat[k*128+p, half*512+col]
    w2   [128, 8, 64]  bf16  per m-tile block-diag: 2 words' [64,32] blocks
    w3   [64, 8, 2]    bf16  per m-tile block-diag: 2 words' [32] columns
    b1   [128, 8] f32, b2 [64, 8] f32, b3 [2, 8] f32 (bias per m-tile col)
    out  [1, 1024] f32
"""

import os

import numpy as np
import ml_dtypes

import concourse.bass as bass
import concourse.tile as tile
from concourse import bacc
from concourse import mybir
from concourse.bass import ts
from concourse.bass_utils import run_bass_kernel_spmd

BF16 = mybir.dt.bfloat16
F32 = mybir.dt.float32
AF = mybir.ActivationFunctionType

N_CORES = 8
B = 8192            # total boxes
BC = B // N_CORES   # boxes per core (1024)
F = 4096            # features
NW = 16             # active words
H1 = 64
H2 = 32
KT = F // 128       # 32 k-tiles
MT = NW * H1 // 128  # 8 m-tiles (wh = w*64+h, 2 words per tile)
NT = BC // 512      # 2 n-tiles of 512 boxes

LAST_RESULTS = None  # BassKernelResults of the most recent run (for test.py)


def build_nc():
    nc = bacc.Bacc("TRN2", target_bir_lowering=False, debug=False)

    xT_d = nc.dram_tensor("xT", [128, KT, BC], BF16, kind="ExternalInput")
    w1_d = nc.dram_tensor("w1", [2, 128, KT, 512], BF16, kind="ExternalInput")
    w2_d = nc.dram_tensor("w2", [128, MT, H1], BF16, kind="ExternalInput")
    w3_d = nc.dram_tensor("w3", [64, MT, 2], BF16, kind="ExternalInput")
    b1_d = nc.dram_tensor("b1", [128, MT], F32, kind="ExternalInput")
    b2_d = nc.dram_tensor("b2", [64, MT], F32, kind="ExternalInput")
    b3_d = nc.dram_tensor("b3", [2, MT], F32, kind="ExternalInput")
    out_d = nc.dram_tensor("out", [1, BC], F32, kind="ExternalOutput")

    with tile.TileContext(nc) as tc:
        with (
            tc.tile_pool(name="big", bufs=1) as big,
            tc.tile_pool(name="smalls", bufs=1) as smalls,
            tc.tile_pool(name="h1p", bufs=4) as h1p,
            tc.tile_pool(name="h2p", bufs=4) as h2p,
            tc.tile_pool(name="sigp", bufs=3) as sigp,
            tc.tile_pool(name="prodp", bufs=1) as prodp,
            tc.tile_pool(name="accp", bufs=6, space="PSUM") as accp,
            tc.tile_pool(name="ps2p", bufs=1, space="PSUM") as ps2p,
            tc.tile_pool(name="ps3p", bufs=1, space="PSUM") as ps3p,
        ):
            w2_sb = smalls.tile([128, MT, H1], BF16, tag="w2", name="w2_sb")
            w3_sb = smalls.tile([64, MT, 2], BF16, tag="w3", name="w3_sb")
            b1_sb = smalls.tile([128, MT], F32, tag="b1", name="b1_sb")
            b2_sb = smalls.tile([64, MT], F32, tag="b2", name="b2_sb")
            b3_sb = smalls.tile([2, MT], F32, tag="b3", name="b3_sb")

            xT_sb = big.tile([128, KT, BC], BF16, tag="xT", name="xT_sb")
            w1_sb = big.tile([128, 2, KT, 512], BF16, tag="w1", name="w1_sb")

            # Chunked loads, 4 k-tiles per chunk (xT 1 MiB / w1-half 0.5 MiB,
            # per-partition runs of 8/4 KiB - big enough that descriptor
            # generation outruns the wire). Interleave so the (mg0, n0)
            # k-loop starts after just the first 1.5 MiB; w1's second half
            # arrives while mg0 computes. Small constants ride along after
            # the first wave (not needed until the first epilogue).
            # Small first waves so the k-loop starts after ~0.75 MiB, then
            # 4-kt chunks: big enough runs (8 KiB xT / 4 KiB w1 per
            # partition) for descriptor generation, small enough that
            # k-tiles become available close to their DMA arrival.
            waves = [(0, 2), (2, 2), (4, 4), (8, 4), (12, 4), (16, 4),
                     (20, 4), (24, 4), (28, 4)]   # (k0, nk)
            for wi, (k0, nk) in enumerate(waves):
                nc.sync.dma_start(
                    out=w1_sb[:, 0, k0:k0 + nk, :], in_=w1_d[0, :, k0:k0 + nk, :]
                )
                nc.sync.dma_start(
                    out=xT_sb[:, k0:k0 + nk, :], in_=xT_d[:, k0:k0 + nk, :]
                )
                if wi == 0:
                    nc.sync.dma_start(out=w2_sb, in_=w2_d[:])
                    nc.sync.dma_start(out=w3_sb, in_=w3_d[:])
                    nc.sync.dma_start(out=b1_sb, in_=b1_d[:])
                    nc.sync.dma_start(out=b2_sb, in_=b2_d[:])
                    nc.sync.dma_start(out=b3_sb, in_=b3_d[:])
            for c in range(2):
                nc.sync.dma_start(
                    out=w1_sb[:, 1, ts(c, 16), :], in_=w1_d[1, :, ts(c, 16), :]
                )

            # Running product over the 8 word-pairs: prod[p, b] accumulates
            # prod_m sigmoid(logits) for pair-slot p (word 2m+p).
            prod = prodp.tile([2, BC], F32, tag="prod", name="prod")

            def l1_matmul(acc, m, n, k):
                nc.tensor.matmul(
                    acc,
                    w1_sb[:, m // 4, k, ts(m % 4, 128)],
                    xT_sb[:, k, ts(n, 512)],
                    start=(k == 0),
                    stop=(k == KT - 1),
                )

            def epilogue(m, n, acc):
                h1_t = h1p.tile([128, 512], BF16, tag="h1", name=f"h1_{m}_{n}")
                nc.scalar.activation(
                    h1_t, acc, AF.Relu, bias=b1_sb[:, m:m + 1]
                )
                ps2 = ps2p.tile([H1, 512], F32, tag="ps2", name=f"ps2_{m}_{n}")
                nc.tensor.matmul(ps2, w2_sb[:, m, :], h1_t, start=True, stop=True)
                h2_t = h2p.tile([H1, 512], BF16, tag="h2", name=f"h2_{m}_{n}")
                nc.scalar.activation(
                    h2_t, ps2, AF.Relu, bias=b2_sb[:, m:m + 1]
                )
                ps3 = ps3p.tile([2, 512], F32, tag="ps3", name=f"ps3_{m}_{n}")
                nc.tensor.matmul(ps3, w3_sb[:, m, :], h2_t, start=True, stop=True)
                if m == 0:
                    nc.scalar.activation(
                        prod[:, ts(n, 512)], ps3, AF.Sigmoid,
                        bias=b3_sb[:, m:m + 1],
                    )
                else:
                    sig_t = sigp.tile(
                        [2, 512], F32, tag="sig", name=f"sig_{m}_{n}"
                    )
                    nc.scalar.activation(
                        sig_t, ps3, AF.Sigmoid, bias=b3_sb[:, m:m + 1]
                    )
                    nc.vector.tensor_mul(
                        prod[:, ts(n, 512)], prod[:, ts(n, 512)], sig_t
                    )

            # Pass 1 (DMA-paced): k-outer over m=0..3 so each k-tile is
            # consumed as its chunk lands.
            accs = {
                m: accp.tile([128, 512], F32, tag="acc", name=f"acc_p1_{m}")
                for m in range(4)
            }
            for k in range(KT):
                for m in range(4):
                    l1_matmul(accs[m], m, 0, k)
            pending = [(m, 0, accs[m]) for m in range(4)]

            # Remaining passes (everything resident): m-serial k-loops, with
            # each epilogue EMITTED one job late. The PE is in-order, so an
            # epilogue's L2/L3 matmuls placed right after their own k-loop
            # would stall the PE on the ACT relu; placed after the NEXT
            # k-loop the relu has long finished. Drain two pending
            # epilogues per job early on to work off pass 1's backlog.
            jobs = [(m, 1) for m in range(4)] + [(m, 0) for m in range(4, 8)] \
                + [(m, 1) for m in range(4, 8)]
            for m, n in jobs:
                acc = accp.tile([128, 512], F32, tag="acc", name=f"acc_{m}_{n}")
                for k in range(KT):
                    l1_matmul(acc, m, n, k)
                for _ in range(2 if len(pending) > 2 else 1):
                    if pending:
                        epilogue(*pending.pop(0))
                pending.append((m, n, acc))
                if (m, n) == (7, 0):
                    # n0's product finishes mid-kernel; its 2->1 combine can
                    # hide under the n1 jobs instead of the kernel tail.
                    while pending:
                        epilogue(*pending.pop(0))
                    r1a = prodp.tile([1, 512], F32, tag="r1a", name="r1a")
                    nc.sync.dma_start(out=r1a, in_=prod[1:2, 0:512])
                    out_a = prodp.tile([1, 512], F32, tag="outa", name="out_a")
                    nc.vector.tensor_mul(out_a, prod[0:1, 0:512], r1a)
                    nc.sync.dma_start(out=out_d[:, 0:512], in_=out_a)
            while pending:
                epilogue(*pending.pop(0))

            # Combine the two pair-slots for the n1 half: engines can't read
            # base partition 1, so bounce it via SBUF->SBUF DMA.
            r1b = prodp.tile([1, 512], F32, tag="r1b", name="r1b")
            nc.sync.dma_start(out=r1b, in_=prod[1:2, 512:1024])
            out_b = prodp.tile([1, 512], F32, tag="outb", name="out_b")
            nc.vector.tensor_mul(out_b, prod[0:1, 512:1024], r1b)
            nc.sync.dma_start(out=out_d[:, 512:1024], in_=out_b)

    nc.compile()
    return nc


_NC_CACHE = None


def _get_nc():
    global _NC_CACHE
    if _NC_CACHE is None:
        _NC_CACHE = build_nc()
    return _NC_CACHE


def _pack_inputs(x, words, W1, b1, W2, b2, W3, b3):
    bf = ml_dtypes.bfloat16
    words = np.asarray(words).astype(np.int64)

    w1g = np.asarray(W1)[words]                     # [16, 4096, 64]
    w1cat = w1g.transpose(1, 0, 2).reshape(F, NW * H1).astype(bf)  # [4096, 1024]
    # -> [half, p, k, col]: partition-major so each partition's whole
    # k-range is one contiguous DMA run.
    w1p = np.ascontiguousarray(
        w1cat.reshape(KT, 128, 2, 512).transpose(2, 1, 0, 3)
    )                                               # [2, 128, 32, 512]
    b1cat = np.asarray(b1)[words].reshape(NW * H1)  # [1024]
    b1p = np.ascontiguousarray(b1cat.reshape(MT, 128).T).astype(np.float32)

    w2g = np.asarray(W2)[words]                     # [16, 64, 32]
    w2blk = np.zeros((MT, 128, H1), np.float32)
    for t in range(MT):
        w2blk[t, 0:64, 0:32] = w2g[2 * t]
        w2blk[t, 64:128, 32:64] = w2g[2 * t + 1]
    w2p = np.ascontiguousarray(w2blk.transpose(1, 0, 2)).astype(bf)  # [128,8,64]
    b2g = np.asarray(b2)[words]                     # [16, 32]
    b2blk = np.zeros((MT, 64), np.float32)
    for t in range(MT):
        b2blk[t, 0:32] = b2g[2 * t]
        b2blk[t, 32:64] = b2g[2 * t + 1]
    b2p = np.ascontiguousarray(b2blk.T).astype(np.float32)           # [64, 8]

    w3g = np.asarray(W3)[words]                     # [16, 32]
    w3blk = np.zeros((MT, 64, 2), np.float32)
    for t in range(MT):
        w3blk[t, 0:32, 0] = w3g[2 * t]
        w3blk[t, 32:64, 1] = w3g[2 * t + 1]
    w3p = np.ascontiguousarray(w3blk.transpose(1, 0, 2)).astype(bf)  # [64, 8, 2]
    b3g = np.asarray(b3)[words]                     # [16]
    b3blk = b3g.reshape(MT, 2)
    b3p = np.ascontiguousarray(b3blk.T).astype(np.float32)           # [2, 8]

    x = np.asarray(x, dtype=np.float32)
    shared = {"w1": w1p, "w2": w2p, "w3": w3p, "b1": b1p, "b2": b2p, "b3": b3p}
    in_maps = []
    for c in range(N_CORES):
        # [p, k, b] partition-major (one contiguous 64 KiB run per partition)
        xT_c = np.ascontiguousarray(
            x[c * BC:(c + 1) * BC, :].astype(bf).T.reshape(KT, 128, BC)
            .transpose(1, 0, 2)
        )
        in_maps.append({"xT": xT_c, **shared})
    return in_maps


def _enable_trace():
    """Register the axon NTFF profile hook (the image's antenv lacks
    axon_hooks, so boot degraded silently) and disable artifact upload."""
    import sys
    import types
    import antenv
    from concourse import bass_utils as bu

    if "antenv.axon_hooks" not in sys.modules:
        mod = types.ModuleType("antenv.axon_hooks")
        mod._hook = None

        def set_axon_ntff_profile_hook(h):
            mod._hook = h

        def get_axon_ntff_profile_hook():
            return mod._hook

        mod.set_axon_ntff_profile_hook = set_axon_ntff_profile_hook
        mod.get_axon_ntff_profile_hook = get_axon_ntff_profile_hook
        sys.modules["antenv.axon_hooks"] = mod
        antenv.axon_hooks = mod

        from trn_agent_boot.trn_boot import _ntff_profile_via_ctypes

        set_axon_ntff_profile_hook(
            _ntff_profile_via_ctypes("/opt/axon/libaxon_pjrt.so")
        )

    bu.upload_artifacts = lambda tmpdir: tmpdir


def kernel(nBBox, x, words, W1, b1, W2, b2, W3, b3):
    global LAST_RESULTS
    nc = _get_nc()
    in_maps = _pack_inputs(x, words, W1, b1, W2, b2, W3, b3)
    trace = bool(int(os.environ.get("KERNEL_TRACE", "0")))
    if trace:
        _enable_trace()
    res = run_bass_kernel_spmd(
        nc, in_maps, core_ids=list(range(N_CORES)), trace=trace
    )
    LAST_RESULTS = res
    out = np.concatenate(
        [res.results[c]["out"].reshape(BC) for c in range(N_CORES)]
    )
    return out.astype(np.float32)[:, None]
